# revision 8
# baseline (speedup 1.0000x reference)
"""Trainium2 Bass kernel for nn_DocumentGraph (hypergraph attention GNN).

Self-contained: kernel(**inputs) -> np.ndarray [32, 1024, 300] f32.
Shards the 32 docs over 8 NeuronCores (4 docs/core), runs a fused
Bass/Tile kernel per core, gathers the output.

Algorithmic structure (per doc, per layer):
  - edge-softmax factorizes: att_e = mask*p / (mask@p), p = exp(lrelu(c + x@u1)),
    so edge aggregation is ONE masked matmul with p-scaled features; the
    softmax denominator Ze and the edge score se ride along as extra columns.
  - x4/e4 are never materialized (folded into weight vectors on the host).
  - node-softmax numerator: A[e,n] = mask*exp(lrelu(sn[n]+se[e]))
      = mask * ee[e] * max(en[n], g[e]*en_a[n])   (exact identity, a<1)
    built with 2 DVE ops per [128,1024] tile from broadcast rows exp(sn).
  - node aggregation: one matmul with A as lhsT; Z_n via a Ze column
    (A is pre-scaled by 1/Ze so the same matmul yields normalized output).
  - gate MLP: LN + sigmoid reduced to per-partition scalar chains; the output
    is written as proj, then g*(sem-proj) is DMA-accumulated on top.
"""
import os
import numpy as np
import concourse.bass as bass
import concourse.bacc as bacc
import concourse.mybir as mybir
import concourse.tile as tile
from concourse.masks import make_identity
from concourse.bass_utils import run_bass_kernel_spmd

P = 128
B, N, E, F = 32, 1024, 512, 300
V = 50001
NCORES = 8
D = B // NCORES          # docs per core
NCH = N // P             # 8 n-chunks
ECH = E // P             # 4 e-chunks
FCH = 3                  # f-chunks (128,128,44)
KF = [128, 128, 44]
FPAD = 384
ALPHA1, ALPHA2 = 0.1, 0.2
dt = mybir.dt
AF = mybir.ActivationFunctionType
OP = mybir.AluOpType

_CACHE = {}


def _pad_f(M):
    """pad leading (feature) dim 300 -> 384 with zeros."""
    out = np.zeros((FPAD,) + M.shape[1:], np.float32)
    out[:F] = M.astype(np.float32)
    return out


def _build(consts):
    """Build the per-core Bass program. consts: host-computed scalars."""
    KSTAGE = int(os.environ.get("KSTAGE", "3"))
    nc = bacc.Bacc()
    c1, c2, b2p, s_a2g = consts["c1"], consts["c2"], consts["b2p"], consts["s_a2g"]

    idx_d = nc.dram_tensor("idx", [P, D * NCH], dt.int32, kind="ExternalInput")
    ht_d = nc.dram_tensor("ht", [D * E, N], dt.float32, kind="ExternalInput")
    emb_d = nc.dram_tensor("emb", [V, F], dt.float32, kind="ExternalInput")
    wpt_d = nc.dram_tensor("wpt", [FPAD, F], dt.float32, kind="ExternalInput")
    w2t_d = nc.dram_tensor("w2t", [FPAD, F], dt.float32, kind="ExternalInput")
    m1_d = nc.dram_tensor("m1", [FPAD, F], dt.float32, kind="ExternalInput")
    a1bt_d = nc.dram_tensor("a1bt", [FPAD, F], dt.float32, kind="ExternalInput")
    u1_d = nc.dram_tensor("u1", [FPAD, 2], dt.float32, kind="ExternalInput")
    u2_d = nc.dram_tensor("u2", [FPAD, 2], dt.float32, kind="ExternalInput")
    u2c1_d = nc.dram_tensor("u2c1", [P, FCH], dt.float32, kind="ExternalInput")
    u2c2_d = nc.dram_tensor("u2c2", [P, FCH], dt.float32, kind="ExternalInput")
    rows_d = nc.dram_tensor("rows", [3, F], dt.float32, kind="ExternalInput")
    out_d = nc.dram_tensor("out", [D * N, F], dt.float32, kind="ExternalOutput")

    with tile.TileContext(nc) as tc:
        with tc.tile_pool(name="const", bufs=1) as cp, \
             tc.tile_pool(name="persist", bufs=1) as pp, \
             tc.tile_pool(name="work", bufs=1) as wp, \
             tc.tile_pool(name="work2", bufs=2) as wp2, \
             tc.tile_pool(name="work3", bufs=3) as wp3, \
             tc.tile_pool(name="pp_ps", bufs=3, space="PSUM") as ps4, \
             tc.tile_pool(name="pt_ps", bufs=3, space="PSUM") as pst_pool, \
             tc.tile_pool(name="pq_ps", bufs=2, space="PSUM") as ps2:

            # ================= setup (once) =================
            ident = cp.tile([P, P], dt.float32)
            make_identity(nc, ident)
            ones_f = cp.tile([P, P], dt.float32)
            nc.vector.memset(ones_f, 1.0)
            ones_r = cp.tile([P, P], dt.float32r)
            nc.vector.tensor_copy(out=ones_r[:], in_=ones_f[:])
            cb2p = cp.tile([P, 1], dt.float32)
            nc.vector.memset(cb2p, -b2p)
            cc1 = cp.tile([P, 1], dt.float32)
            nc.vector.memset(cc1, c1)
            cc2 = cp.tile([P, 1], dt.float32)
            nc.vector.memset(cc2, c2)

            idx_sb = cp.tile([P, D * NCH], dt.int32)
            nc.sync.dma_start(out=idx_sb[:], in_=idx_d[:, :])

            _wstage_i = [0]

            def load_w(d_tensor, width, name):
                st = cp.tile([P, FCH, width], dt.float32,
                             tag=f"wstage{_wstage_i[0] % 2}")
                _wstage_i[0] += 1
                nc.sync.dma_start(
                    out=st[:], in_=d_tensor.rearrange("(c p) h -> p c h", p=P))
                wr = cp.tile([P, FCH, width], dt.float32r, tag=name)
                nc.vector.tensor_copy(out=wr[:], in_=st[:])
                return wr

            wpt = load_w(wpt_d, F, "wpt")
            w2t = load_w(w2t_d, F, "w2t")
            m1 = load_w(m1_d, F, "m1")
            a1bt = load_w(a1bt_d, F, "a1bt")
            u1w = load_w(u1_d, 2, "u1w")
            u2w = load_w(u2_d, 2, "u2w")

            rows_sb = cp.tile([1, 3, F], dt.float32)
            nc.sync.dma_start(out=rows_sb[:1, :, :],
                              in_=rows_d[:, :].rearrange("r h -> (r h)")[None, :])
            rows_r = cp.tile([1, 3, F], dt.float32r)
            nc.vector.tensor_copy(out=rows_r[:], in_=rows_sb[:])

            ps_a = ps4.tile([P, F], dt.float32, tag="pp")
            nc.tensor.matmul(out=ps_a[:], lhsT=ones_f[:1, :],
                             rhs=rows_sb[:1, 2, :], start=True, stop=True)
            a2gb = cp.tile([P, F], dt.float32)
            nc.vector.tensor_copy(out=a2gb[:], in_=ps_a[:])

            u2cs1 = cp.tile([P, FCH], dt.float32, tag="u2cs1")
            nc.sync.dma_start(out=u2cs1[:], in_=u2c1_d[:, :])
            u2cs2 = cp.tile([P, FCH], dt.float32, tag="u2cs2")
            nc.sync.dma_start(out=u2cs2[:], in_=u2c2_d[:, :])
            u2b1 = cp.tile([P, FCH, P], dt.float32r, tag="u2b1")
            u2b2 = cp.tile([P, FCH, P], dt.float32r, tag="u2b2")
            for c in range(FCH):
                nc.vector.tensor_scalar_mul(
                    out=u2b1[:, c, :], in0=ones_f[:], scalar1=u2cs1[:, c:c + 1])
                nc.vector.tensor_scalar_mul(
                    out=u2b2[:, c, :], in0=ones_f[:], scalar1=u2cs2[:, c:c + 1])

            magic = cp.tile([P, NCH], dt.int32)
            nc.vector.memset(magic, 0x5f3759df)

            def transpose_to(x_nm, xT):
                """x_nm [P, NCH, F] f32 node-major -> xT [P, FCH, N] f32r."""
                for fc in range(FCH):
                    kf = KF[fc]
                    for half in range(2):
                        pst = pst_pool.tile([P, 512], dt.float32, tag="pt")
                        for j in range(4):
                            nb = half * 4 + j
                            nc.tensor.transpose(
                                pst[:kf, j * P:(j + 1) * P],
                                x_nm[:, nb, fc * P:fc * P + kf], ident[:])
                        nc.any.tensor_copy(
                            out=xT[:kf, fc, half * 512:(half + 1) * 512],
                            in_=pst[:kf, :])

            # ================= per-doc pipeline =================
            for d in range(D):
                # ---- loads ----
                x0 = wp.tile([P, NCH, F], dt.float32, tag="x0")
                for c in range(NCH):
                    nc.gpsimd.indirect_dma_start(
                        out=x0[:, c, :], out_offset=None, in_=emb_d[:, :],
                        in_offset=bass.IndirectOffsetOnAxis(
                            ap=idx_sb[:, d * NCH + c:d * NCH + c + 1], axis=0))
                mask_bf = wp2.tile([P, ECH, N], dt.bfloat16, tag="mask")
                nc.gpsimd.dma_start(
                    out=mask_bf[:],
                    in_=ht_d[d * E:(d + 1) * E, :].rearrange("(c p) n -> p c n", p=P))
                maskT = wp.tile([P, NCH, E], dt.bfloat16, tag="maskT")
                for ec in range(ECH):
                    nc.sync.dma_start_transpose(
                        maskT[:, :, ec * P:(ec + 1) * P], mask_bf[:, ec, :])

                # ---- x0T + proj ----
                x0T = wp2.tile([P, FCH, N], dt.float32r, tag="x0T")
                transpose_to(x0, x0T)
                proj_t = wp.tile([P, NCH, F], dt.float32, tag="proj_t")
                for c in range(NCH):
                    psp = ps4.tile([P, F], dt.float32, tag="pp")
                    for fc in range(FCH):
                        nc.tensor.matmul(
                            out=psp[:], lhsT=x0T[:KF[fc], fc, c * P:(c + 1) * P],
                            rhs=wpt[:KF[fc], fc, :], start=(fc == 0), stop=False)
                    nc.tensor.matmul(out=psp[:], lhsT=ones_r[:1, :],
                                     rhs=rows_r[:1, 0, :], start=False, stop=True)
                    nc.any.tensor_copy(out=proj_t[:, c, :], in_=psp[:])
                # ---- two hypergraph attention layers ----
                if KSTAGE < 2:
                    nc.sync.dma_start(
                        out=out_d[d * N:(d + 1) * N, :].rearrange(
                            "(c p) h -> p c h", p=P), in_=proj_t[:])
                    continue
                x1 = wp.tile([P, NCH, F], dt.float32, tag="x1")
                dstage = wp.tile([P, NCH, F], dt.float32, tag="gd")
                x1T = wp.tile([P, FCH, N], dt.float32r, tag="x1T")
                sem_t = wp.tile([P, NCH, F], dt.float32, tag="sem_t")
                rzn2 = None

                layers = (1,) if KSTAGE < 3 else (1, 2)
                for lay in layers:
                    if lay == 1:
                        x_in, x_inT, uw, u2b = x0, x0T, u1w, u2b1
                        alpha, cc = ALPHA1, cc1
                    else:
                        x_in, x_inT, uw, u2b = x1, x1T, u2w, u2b2
                        alpha, cc = ALPHA2, cc2

                    # s-matmuls -> [s1, xv] per n-chunk
                    s_sb = wp.tile([P, NCH, 2], dt.float32, tag="s_sb")
                    for c in range(NCH):
                        pss = ps2.tile([P, 2], dt.float32, tag="pq")
                        for fc in range(FCH):
                            nc.tensor.matmul(
                                out=pss[:], lhsT=x_inT[:KF[fc], fc, c * P:(c + 1) * P],
                                rhs=uw[:KF[fc], fc, :], start=(fc == 0),
                                stop=(fc == FCH - 1))
                        nc.vector.tensor_copy(out=s_sb[:, c, :], in_=pss[:])

                    # p = exp(lrelu(c + s1)) ; pxv = p * xv
                    p_pre = wp.tile([P, NCH], dt.float32, tag="p_pre")
                    _prelu = AF.Relu if os.environ.get("KERNEL_SIM_DEBUG") else AF.Prelu
                    nc.scalar.activation(out=p_pre[:], in_=s_sb[:, :, 0],
                                         func=_prelu, bias=cc[:, :1],
                                         scale=1.0, alpha=alpha)
                    p_sb = wp.tile([P, NCH], dt.float32, tag="p_sb")
                    nc.scalar.activation(out=p_sb[:], in_=p_pre[:], func=AF.Exp)
                    pxv = wp.tile([P, NCH], dt.float32, tag="pxv")
                    nc.vector.tensor_tensor(out=pxv[:], in0=p_sb[:],
                                            in1=s_sb[:, :, 1], op=OP.mult)

                    # y = [p*x | p | p*xv]  (bf16)
                    y_bf = wp.tile([P, NCH, 302], dt.bfloat16, tag="y_bf")
                    for c in range(NCH):
                        if lay == 1:
                            nc.vector.tensor_scalar_mul(
                                out=y_bf[:, c, :F], in0=x_in[:, c, :],
                                scalar1=p_sb[:, c:c + 1])
                        else:
                            psx = ps2.tile([P, F], dt.float32, tag="pq")
                            for fc in range(FCH):
                                nc.tensor.matmul(
                                    out=psx[:],
                                    lhsT=x_inT[:KF[fc], fc, c * P:(c + 1) * P],
                                    rhs=w2t[:KF[fc], fc, :], start=(fc == 0),
                                    stop=(fc == FCH - 1))
                            nc.scalar.mul(out=y_bf[:, c, :F], in_=psx[:],
                                          mul=p_sb[:, c:c + 1])
                    nc.vector.tensor_copy(out=y_bf[:, :, F], in_=p_sb[:])
                    nc.vector.tensor_copy(out=y_bf[:, :, F + 1], in_=pxv[:])

                    # stage-A: edge_all = maskT.T @ y   -> [E, 302]
                    edgeN = wp.tile([P, ECH, 302], dt.bfloat16, tag="edgeN")
                    zs_sb = wp.tile([P, ECH, 2], dt.float32, tag="zs_sb")
                    for ec in range(ECH):
                        pse = ps4.tile([P, 302], dt.float32, tag="pp")
                        for c in range(NCH):
                            nc.tensor.matmul(
                                out=pse[:], lhsT=maskT[:, c, ec * P:(ec + 1) * P],
                                rhs=y_bf[:, c, :], start=(c == 0), stop=(c == NCH - 1))
                        nc.any.tensor_copy(out=edgeN[:, ec, :F], in_=pse[:, :F])
                        nc.vector.tensor_copy(out=zs_sb[:, ec, :],
                                              in_=pse[:, F:F + 2])
                    nc.vector.tensor_copy(out=edgeN[:, :, F], in_=zs_sb[:, :, 0])
                    rze = wp.tile([P, ECH], dt.float32, tag="rze")
                    nc.vector.reciprocal(out=rze[:], in_=zs_sb[:, :, 0])
                    se4 = wp.tile([P, ECH], dt.float32, tag="se4")
                    nc.vector.tensor_tensor(out=se4[:], in0=zs_sb[:, :, 1],
                                            in1=rze[:], op=OP.mult)
                    ee4 = wp.tile([P, ECH], dt.float32, tag="ee4")
                    nc.scalar.activation(out=ee4[:], in_=se4[:], func=AF.Exp)
                    g4 = wp.tile([P, ECH], dt.float32, tag="g4")
                    nc.scalar.activation(out=g4[:], in_=se4[:], func=AF.Exp,
                                         scale=alpha - 1.0)
                    ees = wp.tile([P, ECH], dt.float32, tag="ees")
                    nc.vector.tensor_tensor(out=ees[:], in0=ee4[:], in1=rze[:],
                                            op=OP.mult)
                    gee = wp.tile([P, ECH], dt.float32, tag="gee")
                    nc.vector.tensor_tensor(out=gee[:], in0=g4[:], in1=ees[:],
                                            op=OP.mult)

                    # sn broadcast + exp rows (en = exp(sn), enp = exp(alpha*sn))
                    en_bf = wp.tile([P, N], dt.bfloat16, tag="en_bf")
                    enp_bf = wp.tile([P, N], dt.bfloat16, tag="enp_bf")
                    for half in range(2):
                        psb = ps2.tile([P, 512], dt.float32, tag="pq")
                        for fc in range(FCH):
                            nc.tensor.matmul(
                                out=psb[:],
                                lhsT=u2b[:KF[fc], fc, :],
                                rhs=x_inT[:KF[fc], fc, half * 512:(half + 1) * 512],
                                start=(fc == 0), stop=(fc == FCH - 1))
                        nc.scalar.activation(
                            out=en_bf[:, half * 512:(half + 1) * 512], in_=psb[:],
                            func=AF.Exp)
                        nc.scalar.activation(
                            out=enp_bf[:, half * 512:(half + 1) * 512], in_=psb[:],
                            func=AF.Exp, scale=alpha)

                    # A matrix (edge-major, bf16, pre-scaled by 1/Ze)
                    a_bf = wp.tile([P, ECH, N], dt.bfloat16, tag="a_bf")
                    for ec in range(ECH):
                        t1 = wp2.tile([P, N], dt.bfloat16, tag="inner")
                        nc.vector.tensor_scalar_mul(out=t1[:], in0=en_bf[:],
                                                    scalar1=ees[:, ec:ec + 1])
                        t2 = wp2.tile([P, N], dt.bfloat16, tag="inner2")
                        nc.vector.tensor_scalar_mul(out=t2[:], in0=enp_bf[:],
                                                    scalar1=gee[:, ec:ec + 1])
                        nc.vector.tensor_tensor(out=t1[:], in0=t1[:], in1=t2[:],
                                                op=OP.max)
                        nc.vector.tensor_tensor(out=a_bf[:, ec, :], in0=t1[:],
                                                in1=mask_bf[:, ec, :], op=OP.mult)

                    # stage-C node aggregation (node-major)
                    rzn = wp.tile([P, NCH], dt.float32, tag=f"rzn{lay}")
                    for c in range(NCH):
                        psn = ps4.tile([P, 301], dt.float32, tag="pp")
                        for ec in range(ECH):
                            nc.tensor.matmul(
                                out=psn[:], lhsT=a_bf[:, ec, c * P:(c + 1) * P],
                                rhs=edgeN[:, ec, :301], start=(ec == 0),
                                stop=(ec == ECH - 1))
                        nc.vector.reciprocal(out=rzn[:, c:c + 1], in_=psn[:, F:F + 1])
                        if lay == 1:
                            # x1 = elu(node/Zn) = min(max(x,0), e^x - 1)
                            ex = wp3.tile([P, F], dt.float32, tag="ex")
                            nc.scalar.activation(out=ex[:], in_=psn[:, :F],
                                                 func=AF.Exp, scale=rzn[:, c:c + 1])
                            tm = wp3.tile([P, F], dt.float32, tag="tm")
                            nc.scalar.activation(out=tm[:], in_=psn[:, :F],
                                                 func=AF.Relu, scale=rzn[:, c:c + 1])
                            nc.vector.scalar_tensor_tensor(
                                out=x1[:, c, :], in0=ex[:], scalar=-1.0,
                                in1=tm[:], op0=OP.add, op1=OP.min)
                        else:
                            nc.scalar.mul(out=sem_t[:, c, :], in_=psn[:, :F],
                                          mul=rzn[:, c:c + 1])
                            nc.gpsimd.tensor_sub(
                                out=dstage[:, c, :],
                                in0=sem_t[:, c, :], in1=proj_t[:, c, :])
                    if lay == 1:
                        transpose_to(x1, x1T)
                    else:
                        rzn2 = rzn

                # ---- gate pipeline: h_lin -> tanh -> stats ----
                if KSTAGE < 3:
                    nc.sync.dma_start(
                        out=out_d[d * N:(d + 1) * N, :].rearrange(
                            "(c p) h -> p c h", p=P), in_=proj_t[:])
                    continue
                semT = wp.tile([P, FCH, N], dt.float32r, tag="semT")
                transpose_to(sem_t, semT)
                q8 = wp.tile([P, NCH], dt.float32, tag="q8")
                ts8 = wp.tile([P, NCH], dt.float32, tag="ts8")
                tq8 = wp.tile([P, NCH], dt.float32, tag="tq8")
                for c in range(NCH):
                    ph = ps4.tile([P, F], dt.float32, tag="pp")
                    for fc in range(FCH):
                        nc.tensor.matmul(
                            out=ph[:], lhsT=x0T[:KF[fc], fc, c * P:(c + 1) * P],
                            rhs=m1[:KF[fc], fc, :], start=(fc == 0), stop=False)
                    for fc in range(FCH):
                        nc.tensor.matmul(
                            out=ph[:], lhsT=semT[:KF[fc], fc, c * P:(c + 1) * P],
                            rhs=a1bt[:KF[fc], fc, :], start=False, stop=False)
                    nc.tensor.matmul(out=ph[:], lhsT=ones_r[:1, :],
                                     rhs=rows_r[:1, 1, :], start=False, stop=True)
                    th = wp3.tile([P, F], dt.float32, tag="th")
                    nc.scalar.activation(out=th[:], in_=ph[:], func=AF.Tanh,
                                         accum_out=ts8[:, c:c + 1])
                    sq = wp3.tile([P, F], dt.float32, tag="junk")
                    nc.scalar.activation(out=sq[:], in_=th[:], func=AF.Square,
                                         accum_out=tq8[:, c:c + 1])
                    ttr_dst = wp3.tile([P, F], dt.float32, tag="junk")
                    nc.vector.tensor_tensor(out=ttr_dst[:], in0=th[:],
                                            in1=a2gb[:], op=OP.mult)
                    nc.vector.reduce_sum(out=q8[:, c:c + 1], in_=ttr_dst[:],
                                         axis=mybir.AxisListType.X)

                # ---- LN + sigmoid gate (batched, all in exp-set / DVE) ----
                mean8 = wp.tile([P, NCH], dt.float32, tag="mean8")
                nc.vector.tensor_scalar_mul(out=mean8[:], in0=ts8[:],
                                            scalar1=1.0 / F)
                msq = wp.tile([P, NCH], dt.float32, tag="msq")
                nc.vector.tensor_tensor(out=msq[:], in0=mean8[:], in1=mean8[:],
                                        op=OP.mult)
                vpe = wp.tile([P, NCH], dt.float32, tag="vpe")
                nc.vector.scalar_tensor_tensor(
                    out=vpe[:], in0=tq8[:], scalar=1.0 / F, in1=msq[:],
                    op0=OP.mult, op1=OP.subtract)
                nc.vector.tensor_scalar_add(out=vpe[:], in0=vpe[:], scalar1=1e-5)
                # rstd = rsqrt(vpe): bit-trick + 3 Newton iterations
                rstd = wp.tile([P, NCH], dt.float32, tag="rstd")
                sh_i = wp.tile([P, NCH], dt.int32, tag="sh_i")
                nc.vector.tensor_scalar(out=sh_i[:], in0=vpe[:].bitcast(dt.int32),
                                        scalar1=1, scalar2=None,
                                        op0=OP.logical_shift_right)
                nc.vector.tensor_tensor(out=rstd[:].bitcast(dt.int32), in0=magic[:],
                                        in1=sh_i[:], op=OP.subtract)
                nt1 = wp.tile([P, NCH], dt.float32, tag="nt1")
                for _ in range(3):
                    nc.vector.tensor_tensor(out=nt1[:], in0=rstd[:], in1=rstd[:],
                                            op=OP.mult)
                    nc.vector.tensor_tensor(out=nt1[:], in0=nt1[:], in1=vpe[:],
                                            op=OP.mult)
                    nc.vector.tensor_scalar(out=nt1[:], in0=nt1[:], scalar1=-0.5,
                                            scalar2=1.5, op0=OP.mult, op1=OP.add)
                    nc.vector.tensor_tensor(out=rstd[:], in0=rstd[:], in1=nt1[:],
                                            op=OP.mult)
                gp8 = wp.tile([P, NCH], dt.float32, tag="gp8")
                nc.vector.scalar_tensor_tensor(
                    out=gp8[:], in0=mean8[:], scalar=-s_a2g, in1=q8[:],
                    op0=OP.mult, op1=OP.add)
                nc.vector.tensor_tensor(out=gp8[:], in0=gp8[:], in1=rstd[:],
                                        op=OP.mult)
                eneg = wp.tile([P, NCH], dt.float32, tag="eneg")
                nc.scalar.activation(out=eneg[:], in_=gp8[:], func=AF.Exp,
                                     bias=cb2p[:, :1], scale=-1.0)
                gate8 = wp.tile([P, NCH], dt.float32, tag="gate8")
                nc.vector.tensor_scalar_add(out=gate8[:], in0=eneg[:], scalar1=1.0)
                nc.vector.reciprocal(out=gate8[:], in_=gate8[:])

                # out = proj + gate*(sem - proj)
                gfin = wp.tile([P, NCH, F], dt.float32, tag="gfin")
                for c in range(NCH):
                    nc.vector.scalar_tensor_tensor(
                        out=gfin[:, c, :], in0=dstage[:, c, :],
                        scalar=gate8[:, c:c + 1], in1=proj_t[:, c, :],
                        op0=OP.mult, op1=OP.add)
                nc.sync.dma_start(
                    out=out_d[d * N:(d + 1) * N, :].rearrange("(c p) h -> p c h", p=P),
                    in_=gfin[:])

    nc.finalize()
    return nc


def prepare_in_maps(inputs, HT, emb, Wp, bp, w2_1, w3_1, wc1, a11, a21,
                    W2t, w2_2, w3_2, wc2, a12, a22, A1, b1, ln_g, ln_b, A2, b2):
    f32 = np.float32
    inputs = np.asarray(inputs)
    HT = np.asarray(HT, f32)
    emb = np.ascontiguousarray(np.asarray(emb, f32))
    Wp, bp = np.asarray(Wp, f32), np.asarray(bp, f32)
    W2t = np.asarray(W2t, f32)
    w2_1, w3_1 = np.asarray(w2_1, f32), np.asarray(w3_1, f32)
    w2_2, w3_2 = np.asarray(w2_2, f32), np.asarray(w3_2, f32)
    a11, a21 = np.asarray(a11, f32), np.asarray(a21, f32)
    a12, a22 = np.asarray(a12, f32), np.asarray(a22, f32)
    A1, b1 = np.asarray(A1, f32), np.asarray(b1, f32)
    A2, b2 = np.asarray(A2, f32), np.asarray(b2, f32)
    ln_g, ln_b = np.asarray(ln_g, f32), np.asarray(ln_b, f32)
    wc1, wc2 = np.asarray(wc1, f32), np.asarray(wc2, f32)

    A1a, A1b = A1[:, :F], A1[:, F:]
    consts = {
        "c1": float(wc1 @ a11[:F]),
        "c2": float(wc2 @ a12[:F]),
        "b2p": float(b2[0] + ln_b @ A2[0]),
        "s_a2g": float(np.sum(ln_g * A2[0])),
    }
    a2g = (ln_g * A2[0]).astype(f32)
    b1p = (b1 + bp @ A1a.T).astype(f32)
    rows = np.stack([bp, b1p, a2g]).astype(f32)

    # folded score vectors: col0 = u1 (edge score), col1 = uv (se feature dot)
    uL1 = np.stack([w2_1 @ a11[F:], w3_1 @ a21[F:]], axis=1)
    uL2 = np.stack([w2_2 @ a12[F:], W2t @ (w3_2 @ a22[F:])], axis=1)
    u2v1 = w2_1 @ a21[:F]   # sn vector, layer 1
    u2v2 = w2_2 @ a22[:F]   # sn vector, layer 2

    def u2cols(v):
        vp = np.zeros((FPAD,), f32)
        vp[:F] = v
        return np.ascontiguousarray(vp.reshape(FCH, P).T)

    shared = {
        "emb": emb,
        "wpt": _pad_f(Wp.T),
        "w2t": _pad_f(W2t),
        "m1": _pad_f((A1a @ Wp).T),
        "a1bt": _pad_f(A1b.T),
        "u1": _pad_f(uL1),
        "u2": _pad_f(uL2),
        "u2c1": u2cols(u2v1),
        "u2c2": u2cols(u2v2),
        "rows": rows,
    }

    idx = inputs.astype(np.int32)
    in_maps = []
    for k in range(NCORES):
        idx_k = np.ascontiguousarray(
            idx[D * k:D * (k + 1)].reshape(D * NCH, P).T)
        ht_k = np.ascontiguousarray(HT[D * k:D * (k + 1)].reshape(D * E, N))
        in_maps.append({"idx": idx_k, "ht": ht_k, **shared})
    return {"_consts": consts, "in_maps": in_maps}


LAST_RESULT = None


def kernel(**inputs):
    global LAST_RESULT
    maps = prepare_in_maps(**inputs)
    consts, in_maps = maps["_consts"], maps["in_maps"]
    key = tuple(sorted(consts.items()))
    if key not in _CACHE:
        _CACHE[key] = _build(consts)
    nc = _CACHE[key]
    res = run_bass_kernel_spmd(nc, in_maps, core_ids=list(range(NCORES)))
    LAST_RESULT = res
    out = np.empty((B, N, F), np.float32)
    for k in range(NCORES):
        out[D * k:D * (k + 1)] = res.results[k]["out"].reshape(D, N, F)
    return out

